# revision 18
# baseline (speedup 1.0000x reference)
"""Trainium2 Bass kernel for nn_SAW_53395033424216 (grouped-covariance loss).

Math (see reference): for each sample b and channel-group g (16 channels),
  cov[b,g] = (Xg Xg^T)/(HW-1) with Xg rows scaled by wgh; loss is the
  mean-over-B sum-over-g of the masked (strict upper triangle) abs-sum of
  cov / num_off.

Strategy:
  * Host: compute perm/wgh from classifier_w (tiny), permute channels so each
    group is 16 consecutive channels, transpose each sample to [HW, 512] and
    cast to bf16 (the 61k-entry abs-sum averages away bf16 noise; measured
    rel-err ~2.4e-6 on the fixed seed inputs).
  * Device (8 cores, 2 samples each): stream [128hw x 512ch] bf16 tiles;
    for each 128-channel block (= 8 whole groups) accumulate the 128x128
    Gram matrix over all 16384 hw rows via PE matmuls (contraction on
    partitions). Weight-scaling is bilinear -> folded into a per-block
    [128,128] mask/weight tile applied once at the end (DVE), followed by
    an abs row-reduce. Output: [128,1] partial sums per core.
  * Host: sum partials -> loss (clamp is a no-op since summands are >= 0).
"""

import os

# Whole-tile dependency tracking only: with per-subtile releases the slab DMA
# accumulates more sync-waits than the DMA pseudo-instruction format allows
# ("Too many sync wait commands" in walrus codegen).
os.environ.setdefault("BY_DEFAULT_DISABLE_SUBTILE_DEPS", "1")

import numpy as np
import ml_dtypes

import concourse.bass as bass
import concourse.mybir as mybir
from concourse.tile import TileContext
from concourse.bass_utils import run_bass_kernel_spmd

# Problem constants (hardcoded per the harness contract)
B = 16          # batch
CH = 512        # channels
H = W = 128
HW = H * W      # 16384
C = 16          # selected classes = group width
G = CH // C     # 32 groups
N_CORES = 8
SAMPLES_PER_CORE = B // N_CORES  # 2
NUM_OFF = C * (C - 1) // 2       # 120

# Data dtype on the wire/PE: bfloat16 (rel err ~2.4e-6) or float8_e4m3
# (rel err ~8.1e-4, half the DMA traffic).
DATA_DT_NAME = "float8e4"
SLAB = 4        # hw-chunks per DMA; small slabs let the first matmuls start early
N_WARMUP_MM = 28  # dummy matmuls during the initial DMA wait to lift the PE HAM throttle
N_CHUNKS = HW // 128             # 128
N_SLABS = N_CHUNKS // SLAB       # 16
N_CB = CH // 128                 # 4 channel blocks

_PROGRAM = None
LAST_RESULTS = None  # BassKernelResults of the most recent run (for test.py)


def _ensure_ntff_hook():
    """Provide antenv.axon_hooks if the image lacks it, so BASS_TRACE=1
    profiling works under axon (drives NTFF capture via the axon PJRT .so)."""
    try:
        import antenv.axon_hooks  # noqa: F401

        return
    except ImportError:
        pass
    import contextlib
    import ctypes
    import sys
    import types

    try:
        import antenv
    except ImportError:
        return

    so_path = "/opt/axon/libaxon_pjrt.so"
    if not os.path.exists(so_path):
        return
    lib = ctypes.CDLL(so_path)
    if not hasattr(lib, "axon_start_nrt_profile"):
        hook = None
    else:
        lib.axon_start_nrt_profile.argtypes = [
            ctypes.POINTER(ctypes.c_int64),
            ctypes.c_size_t,
        ]
        lib.axon_start_nrt_profile.restype = ctypes.c_int64
        lib.axon_stop_nrt_profile.argtypes = [ctypes.c_char_p]
        lib.axon_stop_nrt_profile.restype = ctypes.c_int64

        @contextlib.contextmanager
        def hook(output_dir, device_ids):
            import jax

            jax.devices()  # ensure the PJRT client exists before start
            if device_ids:
                ids = (ctypes.c_int64 * len(device_ids))(*device_ids)
                rc = lib.axon_start_nrt_profile(ids, len(device_ids))
            else:
                rc = lib.axon_start_nrt_profile(None, 0)
            if rc != 0:
                raise RuntimeError(f"axon_start_nrt_profile rc={rc}")
            try:
                yield
            finally:
                n = lib.axon_stop_nrt_profile(str(output_dir).encode())
                if n < 0:
                    raise RuntimeError(f"axon_stop_nrt_profile rc={n}")

    state = {"hook": hook}
    mod = types.ModuleType("antenv.axon_hooks")
    mod.get_axon_ntff_profile_hook = lambda: state["hook"]
    mod.set_axon_ntff_profile_hook = lambda h: state.update(hook=h)
    sys.modules["antenv.axon_hooks"] = mod
    antenv.axon_hooks = mod


_ensure_ntff_hook()


def _build_program():
    nc = bass.Bass()
    f32 = mybir.dt.float32
    data_dt = getattr(mybir.dt, DATA_DT_NAME)

    # Host pre-tiled layout: [s, slab, partition, k, c] so each partition's
    # slab slice is one contiguous 8 KiB run in DRAM (max DMA efficiency).
    xt = nc.dram_tensor(
        "xt", [SAMPLES_PER_CORE, N_SLABS, 128, SLAB, CH], data_dt, kind="ExternalInput"
    )
    wm = nc.dram_tensor("wm", [N_CB, 128, 128], f32, kind="ExternalInput")
    out = nc.dram_tensor("out", [128, 1], f32, kind="ExternalOutput")

    with TileContext(nc) as tc:
        with (
            tc.tile_pool(name="wpool", bufs=1) as wpool,
            tc.tile_pool(name="data", bufs=16) as dpool,
            tc.tile_pool(name="scratch", bufs=2) as spool,
            tc.tile_pool(name="redp", bufs=1) as redp,
            tc.tile_pool(name="psum", bufs=2, space="PSUM") as psum_pool,
        ):
            wm_t = wpool.tile([128, N_CB, 128], f32)
            nc.sync.dma_start(out=wm_t, in_=wm.transpose([1, 0, 2]))

            red_all = redp.tile([128, SAMPLES_PER_CORE * N_CB], f32)

            # PE warm-up: ~3us of throwaway matmuls while the first data slab
            # is still in flight, so the HAM clock gate reaches 8/8 before the
            # real stream begins.  Shares the gram0 slot tag; real use of that
            # slot starts with start=True which clears it.
            warm_in = wpool.tile([128, 128], data_dt, name="warm_in")
            nc.vector.memset(warm_in, 1)
            warm_ps = psum_pool.tile([128, 128], f32, name="warm_ps", tag="gram0")
            for _ in range(N_WARMUP_MM):
                nc.tensor.matmul(
                    warm_ps[:, :], lhsT=warm_in, rhs=warm_in, start=True, stop=True
                )

            for s in range(SAMPLES_PER_CORE):
                grams = [
                    psum_pool.tile(
                        [128, 128], f32, name=f"gram{cb}", tag=f"gram{cb}"
                    )
                    for cb in range(N_CB)
                ]
                for sl in range(N_SLABS):
                    dt_t = dpool.tile([128, SLAB, CH], data_dt)
                    nc.sync.dma_start(out=dt_t, in_=xt[s, sl])
                    for k in range(SLAB):
                        h = sl * SLAB + k
                        for cb in range(N_CB):
                            t = dt_t[:, k, cb * 128 : (cb + 1) * 128]
                            nc.tensor.matmul(
                                grams[cb][:, :],
                                lhsT=t,
                                rhs=t,
                                start=(h == 0),
                                stop=(h == N_CHUNKS - 1),
                            )
                # post-process: red[:, s*N_CB+cb] = sum_j |gram_ij| * wm[cb,i,j]
                for cb in range(N_CB):
                    scr = spool.tile([128, 128], f32)
                    nc.vector.tensor_mul(scr, grams[cb][:, :], wm_t[:, cb, :])
                    idx = s * N_CB + cb
                    nc.vector.tensor_reduce(
                        out=red_all[:, idx : idx + 1],
                        in_=scr,
                        axis=mybir.AxisListType.X,
                        op=mybir.AluOpType.add,
                        apply_absolute_value=True,
                    )

            racc = spool.tile([128, 1], f32, tag="racc")
            nc.vector.tensor_reduce(
                out=racc,
                in_=red_all,
                axis=mybir.AxisListType.X,
                op=mybir.AluOpType.add,
            )
            nc.sync.dma_start(out=out[:, :], in_=racc)

    _reduce_sync_waits(nc)
    return nc


# Procs whose semaphores advance in instruction (program) order.  DMA lanes
# qualify: each lane's DMAs go through the same FIFO ring and complete (inc
# their lane sem) in issue order per SDMA engine.  GpSimd (Pool) does not
# (8 independent Q7 FIFOs) - we never emit Pool work.
_INORDER = ("PE", "DVE", "Activation", "SP", "DMAHW", "DMASW")


def _reduce_sync_waits(nc):
    """Walrus' per-instruction sync-wait capacity is 1 for DMA/compute
    pseudo-instructions (and small for Drain), but Tile's semaphore pass is
    not transitively minimal and can emit more. Reduce every wait list to
    its weakest sufficient single wait by proving the rest redundant:

    (a) waits on the instruction's own in-order proc sem are implied by
        stream position;
    (b) for each candidate kept wait (sem_k >= v_k): every other wait
        (sem_d >= v_d) must hold once sem_k reaches v_k.  That holds if an
        instruction at-or-before tick v_k in sem_k's stream carried
        (transitively) a wait implying it -- sems are monotone, so a wait
        that held once holds forever.
    """
    insts = [i for fn in nc.m.functions for blk in fn.blocks for i in blk.instructions]

    def proc_of_sem(name):
        return name.rsplit("_", 1)[0]  # e.g. "DMAHW3_44" -> "DMAHW3"

    # Per proc: ordered stream of (waits, cumulative-sem-value-after).
    streams = {}
    # Per instruction id: [(proc, sem-value-before-this-instruction)]
    positions = {}

    def add_to_stream(inst, proc, waits, upd):
        lst = streams.setdefault(proc, [])
        prev = lst[-1][1] if lst else 0
        positions.setdefault(id(inst), []).append((proc, prev))
        lst.append((waits, prev + upd))

    eng_sem = {"PE": "PE", "DVE": "DVE", "ACT": "Activation", "SP": "SP"}
    for inst in insts:
        si = inst.sync_info
        waits = [(w.ant_name, w.wait_value) for w in si.on_wait] if si else []
        if type(inst).__name__ == "InstDMACopy":
            # completion updates belong to the DMA lane proc
            for u in si.on_update:
                add_to_stream(inst, proc_of_sem(u.ant_name), waits, u.update_value)
        else:
            en = str(inst.engine).split(".")[-1]
            pref = eng_sem.get(en)
            if pref is None:
                continue
            upd = 0
            if si:
                for u in si.on_update:
                    if proc_of_sem(u.ant_name) == pref:
                        upd += u.update_value
            add_to_stream(inst, pref, waits, upd)

    from functools import lru_cache

    @lru_cache(maxsize=None)
    def holds(proc, tick, sem_d, v_d, depth=4):
        """Once `proc`'s sem has reached `tick`, does sem_d >= v_d hold?

        Covered prefix: entries up to the last one whose own completion is
        certified (cumulative sem value <= tick) have issued, so their waits
        held at some past moment; sems are monotone, so they hold now.
        """
        if proc == proc_of_sem(sem_d):
            return tick >= v_d
        if depth == 0 or not proc.startswith(_INORDER):
            return False
        stream = streams.get(proc, [])
        last = -1
        prev = 0
        for i, (waits, cum) in enumerate(stream):
            if cum > tick:
                break
            if cum > prev:
                last = i  # completing instruction within budget
            prev = cum
        for waits, _cum in stream[: last + 1]:
            for (s, v) in waits:
                if s == sem_d and v >= v_d:
                    return True
                if holds(proc_of_sem(s), v, sem_d, v_d, depth - 1):
                    return True
        return False

    for inst in insts:
        tn = type(inst).__name__
        si = inst.sync_info
        if si is None or len(si.on_wait) <= 1:
            continue
        # Drop waits implied by the instruction's own position in its
        # in-order stream(s): at least `v` completions of that proc precede
        # it in program order.
        own = [
            (proc, prefix)
            for proc, prefix in positions.get(id(inst), [])
            if proc.startswith(_INORDER)
        ]
        kept_sw = []
        for w in si.on_wait:
            wp = proc_of_sem(w.ant_name)
            if any(proc == wp and prefix >= w.wait_value for proc, prefix in own):
                continue
            kept_sw.append(w)
        if len(kept_sw) <= 1:
            si.on_wait = kept_sw
            continue
        waits = [(w.ant_name, w.wait_value) for w in kept_sw]
        chosen = None
        for k, (sem_k, v_k) in enumerate(waits):
            if not proc_of_sem(sem_k).startswith(_INORDER):
                continue
            if all(
                holds(proc_of_sem(sem_k), v_k, sem_d, v_d)
                for d, (sem_d, v_d) in enumerate(waits)
                if d != k
            ):
                chosen = k
                break
        assert chosen is not None, (
            f"{inst.name} ({tn}): cannot reduce waits to 1: {waits}"
        )
        si.on_wait = [kept_sw[chosen]]


def _host_prep(x, classifier_w, sel):
    """Compute perm / per-block weight-mask and the per-core bf16 shards."""
    x = np.asarray(x)
    w = np.asarray(classifier_w).astype(np.float32)
    sel = np.asarray(sel).astype(np.int64)

    w_abs = np.abs(w)
    idx = np.argsort(-w_abs, axis=1, kind="stable")  # matches jnp.argsort (stable)
    sig = (1.0 / (1.0 + np.exp(-w_abs.astype(np.float64)))).astype(np.float32)

    idx_sel = idx[sel]               # [C, CH]
    ch_ids = idx_sel[:, :G].T        # [G, C]
    perm = ch_ids.reshape(G * C)     # output channel g*C+c <- input channel
    wgh = sig[sel[None, :], ch_ids].reshape(G * C).astype(np.float64)

    # Per-channel-block weight/mask tile, with all scalar factors folded in:
    # wm[cb, i, j] = wgh_i * wgh_j * [same 16-group, j > i] / ((HW-1)*NUM_OFF*B)
    wm = np.zeros((N_CB, 128, 128), dtype=np.float64)
    scale = 1.0 / ((HW - 1) * NUM_OFF * B)
    ii, jj = np.meshgrid(np.arange(128), np.arange(128), indexing="ij")
    blockmask = ((ii // C) == (jj // C)) & (jj > ii)
    for cb in range(N_CB):
        wloc = wgh[cb * 128 : (cb + 1) * 128]
        wm[cb] = np.outer(wloc, wloc) * blockmask * scale
    wm = wm.astype(np.float32)

    # Per-core shards: samples [2c, 2c+1] -> permuted channels, hw-major,
    # pre-tiled as [s, slab, partition, k, c] so each partition's slab row is
    # one contiguous 8 KiB DRAM run.
    xr = x.reshape(B, CH, HW)
    shards = []
    for c in range(N_CORES):
        xs = xr[c * SAMPLES_PER_CORE : (c + 1) * SAMPLES_PER_CORE][:, perm, :]
        np_dt = mybir.dt.np(getattr(mybir.dt, DATA_DT_NAME))
        xb = xs.transpose(0, 2, 1).astype(np_dt)  # [S, HW, CH]
        xt = np.ascontiguousarray(
            xb.reshape(SAMPLES_PER_CORE, N_SLABS, SLAB, 128, CH).transpose(
                0, 1, 3, 2, 4
            )
        )
        shards.append(xt)
    return shards, wm


def kernel(x, classifier_w, sel):
    global _PROGRAM, LAST_RESULTS
    assert x.shape == (B, CH, H, W), x.shape

    shards, wm = _host_prep(x, classifier_w, sel)

    if _PROGRAM is None:
        _PROGRAM = _build_program()

    in_maps = [{"xt": shards[c], "wm": wm} for c in range(N_CORES)]
    LAST_RESULTS = run_bass_kernel_spmd(_PROGRAM, in_maps, core_ids=list(range(N_CORES)))

    total = np.float64(0.0)
    for r in LAST_RESULTS.results:
        total += np.float64(r["out"].sum(dtype=np.float64))
    return np.array([total], dtype=np.float32)


# revision 19
# speedup vs baseline: 1.0295x; 1.0295x over previous
"""Trainium2 Bass kernel for nn_SAW_53395033424216 (grouped-covariance loss).

Math (see reference): for each sample b and channel-group g (16 channels),
  cov[b,g] = (Xg Xg^T)/(HW-1) with Xg rows scaled by wgh; loss is the
  mean-over-B sum-over-g of the masked (strict upper triangle) abs-sum of
  cov / num_off.

Strategy:
  * Host: compute perm/wgh from classifier_w (tiny), permute channels so each
    group is 16 consecutive channels, transpose each sample to [HW, 512] and
    cast to bf16 (the 61k-entry abs-sum averages away bf16 noise; measured
    rel-err ~2.4e-6 on the fixed seed inputs).
  * Device (8 cores, 2 samples each): stream [128hw x 512ch] bf16 tiles;
    for each 128-channel block (= 8 whole groups) accumulate the 128x128
    Gram matrix over all 16384 hw rows via PE matmuls (contraction on
    partitions). Weight-scaling is bilinear -> folded into a per-block
    [128,128] mask/weight tile applied once at the end (DVE), followed by
    an abs row-reduce. Output: [128,1] partial sums per core.
  * Host: sum partials -> loss (clamp is a no-op since summands are >= 0).
"""

import os

# Whole-tile dependency tracking only: with per-subtile releases the slab DMA
# accumulates more sync-waits than the DMA pseudo-instruction format allows
# ("Too many sync wait commands" in walrus codegen).
os.environ.setdefault("BY_DEFAULT_DISABLE_SUBTILE_DEPS", "1")

import numpy as np
import ml_dtypes

import concourse.bass as bass
import concourse.mybir as mybir
from concourse.tile import TileContext
from concourse.bass_utils import run_bass_kernel_spmd

# Problem constants (hardcoded per the harness contract)
B = 16          # batch
CH = 512        # channels
H = W = 128
HW = H * W      # 16384
C = 16          # selected classes = group width
G = CH // C     # 32 groups
N_CORES = 8
SAMPLES_PER_CORE = B // N_CORES  # 2
NUM_OFF = C * (C - 1) // 2       # 120

# Data dtype on the wire/PE: bfloat16 (rel err ~2.4e-6) or float8_e4m3
# (rel err ~8.1e-4, half the DMA traffic).
DATA_DT_NAME = "float8e4"
SLAB = 4        # hw-chunks per DMA; small slabs let the first matmuls start early
N_WARMUP_MM = 52  # dummy matmuls during the initial DMA wait to lift the PE HAM throttle
N_CHUNKS = HW // 128             # 128
N_SLABS = N_CHUNKS // SLAB       # 16
N_CB = CH // 128                 # 4 channel blocks

_PROGRAM = None
LAST_RESULTS = None  # BassKernelResults of the most recent run (for test.py)


def _ensure_ntff_hook():
    """Provide antenv.axon_hooks if the image lacks it, so BASS_TRACE=1
    profiling works under axon (drives NTFF capture via the axon PJRT .so)."""
    try:
        import antenv.axon_hooks  # noqa: F401

        return
    except ImportError:
        pass
    import contextlib
    import ctypes
    import sys
    import types

    try:
        import antenv
    except ImportError:
        return

    so_path = "/opt/axon/libaxon_pjrt.so"
    if not os.path.exists(so_path):
        return
    lib = ctypes.CDLL(so_path)
    if not hasattr(lib, "axon_start_nrt_profile"):
        hook = None
    else:
        lib.axon_start_nrt_profile.argtypes = [
            ctypes.POINTER(ctypes.c_int64),
            ctypes.c_size_t,
        ]
        lib.axon_start_nrt_profile.restype = ctypes.c_int64
        lib.axon_stop_nrt_profile.argtypes = [ctypes.c_char_p]
        lib.axon_stop_nrt_profile.restype = ctypes.c_int64

        @contextlib.contextmanager
        def hook(output_dir, device_ids):
            import jax

            jax.devices()  # ensure the PJRT client exists before start
            if device_ids:
                ids = (ctypes.c_int64 * len(device_ids))(*device_ids)
                rc = lib.axon_start_nrt_profile(ids, len(device_ids))
            else:
                rc = lib.axon_start_nrt_profile(None, 0)
            if rc != 0:
                raise RuntimeError(f"axon_start_nrt_profile rc={rc}")
            try:
                yield
            finally:
                n = lib.axon_stop_nrt_profile(str(output_dir).encode())
                if n < 0:
                    raise RuntimeError(f"axon_stop_nrt_profile rc={n}")

    state = {"hook": hook}
    mod = types.ModuleType("antenv.axon_hooks")
    mod.get_axon_ntff_profile_hook = lambda: state["hook"]
    mod.set_axon_ntff_profile_hook = lambda h: state.update(hook=h)
    sys.modules["antenv.axon_hooks"] = mod
    antenv.axon_hooks = mod


_ensure_ntff_hook()


def _build_program():
    nc = bass.Bass()
    f32 = mybir.dt.float32
    data_dt = getattr(mybir.dt, DATA_DT_NAME)

    # Host pre-tiled layout: [s, slab, partition, k, c] so each partition's
    # slab slice is one contiguous 8 KiB run in DRAM (max DMA efficiency).
    xt = nc.dram_tensor(
        "xt", [SAMPLES_PER_CORE, N_SLABS, 128, SLAB, CH], data_dt, kind="ExternalInput"
    )
    wm = nc.dram_tensor("wm", [N_CB, 128, 128], f32, kind="ExternalInput")
    out = nc.dram_tensor("out", [128, 1], f32, kind="ExternalOutput")

    with TileContext(nc) as tc:
        with (
            tc.tile_pool(name="wpool", bufs=1) as wpool,
            tc.tile_pool(name="data", bufs=16) as dpool,
            tc.tile_pool(name="scratch", bufs=2) as spool,
            tc.tile_pool(name="redp", bufs=1) as redp,
            tc.tile_pool(name="psum", bufs=2, space="PSUM") as psum_pool,
        ):
            wm_t = wpool.tile([128, N_CB, 128], f32)
            nc.sync.dma_start(out=wm_t, in_=wm.transpose([1, 0, 2]))

            red_all = redp.tile([128, SAMPLES_PER_CORE * N_CB], f32)

            # PE warm-up: ~3us of throwaway matmuls while the first data slab
            # is still in flight, so the HAM clock gate reaches 8/8 before the
            # real stream begins.  Shares the gram0 slot tag; real use of that
            # slot starts with start=True which clears it.
            warm_in = wpool.tile([128, 128], data_dt, name="warm_in")
            nc.vector.memset(warm_in, 1)
            warm_ps = psum_pool.tile([128, 128], f32, name="warm_ps", tag="gram0")
            for _ in range(N_WARMUP_MM):
                nc.tensor.matmul(
                    warm_ps[:, :], lhsT=warm_in, rhs=warm_in, start=True, stop=True
                )

            for s in range(SAMPLES_PER_CORE):
                grams = [
                    psum_pool.tile(
                        [128, 128], f32, name=f"gram{cb}", tag=f"gram{cb}"
                    )
                    for cb in range(N_CB)
                ]
                for sl in range(N_SLABS):
                    dt_t = dpool.tile([128, SLAB, CH], data_dt)
                    nc.sync.dma_start(out=dt_t, in_=xt[s, sl])
                    for k in range(SLAB):
                        h = sl * SLAB + k
                        for cb in range(N_CB):
                            t = dt_t[:, k, cb * 128 : (cb + 1) * 128]
                            nc.tensor.matmul(
                                grams[cb][:, :],
                                lhsT=t,
                                rhs=t,
                                start=(h == 0),
                                stop=(h == N_CHUNKS - 1),
                            )
                # post-process: red[:, s*N_CB+cb] = sum_j |gram_ij| * wm[cb,i,j]
                for cb in range(N_CB):
                    scr = spool.tile([128, 128], f32)
                    nc.vector.tensor_mul(scr, grams[cb][:, :], wm_t[:, cb, :])
                    idx = s * N_CB + cb
                    nc.vector.tensor_reduce(
                        out=red_all[:, idx : idx + 1],
                        in_=scr,
                        axis=mybir.AxisListType.X,
                        op=mybir.AluOpType.add,
                        apply_absolute_value=True,
                    )

            racc = spool.tile([128, 1], f32, tag="racc")
            nc.vector.tensor_reduce(
                out=racc,
                in_=red_all,
                axis=mybir.AxisListType.X,
                op=mybir.AluOpType.add,
            )
            nc.sync.dma_start(out=out[:, :], in_=racc, single_packet=True)

    _reduce_sync_waits(nc)
    return nc


# Procs whose semaphores advance in instruction (program) order.  DMA lanes
# qualify: each lane's DMAs go through the same FIFO ring and complete (inc
# their lane sem) in issue order per SDMA engine.  GpSimd (Pool) does not
# (8 independent Q7 FIFOs) - we never emit Pool work.
_INORDER = ("PE", "DVE", "Activation", "SP", "DMAHW", "DMASW")


def _reduce_sync_waits(nc):
    """Walrus' per-instruction sync-wait capacity is 1 for DMA/compute
    pseudo-instructions (and small for Drain), but Tile's semaphore pass is
    not transitively minimal and can emit more. Reduce every wait list to
    its weakest sufficient single wait by proving the rest redundant:

    (a) waits on the instruction's own in-order proc sem are implied by
        stream position;
    (b) for each candidate kept wait (sem_k >= v_k): every other wait
        (sem_d >= v_d) must hold once sem_k reaches v_k.  That holds if an
        instruction at-or-before tick v_k in sem_k's stream carried
        (transitively) a wait implying it -- sems are monotone, so a wait
        that held once holds forever.
    """
    insts = [i for fn in nc.m.functions for blk in fn.blocks for i in blk.instructions]

    def proc_of_sem(name):
        return name.rsplit("_", 1)[0]  # e.g. "DMAHW3_44" -> "DMAHW3"

    # Per proc: ordered stream of (waits, cumulative-sem-value-after).
    streams = {}
    # Per instruction id: [(proc, sem-value-before-this-instruction)]
    positions = {}

    def add_to_stream(inst, proc, waits, upd):
        lst = streams.setdefault(proc, [])
        prev = lst[-1][1] if lst else 0
        positions.setdefault(id(inst), []).append((proc, prev))
        lst.append((waits, prev + upd))

    eng_sem = {"PE": "PE", "DVE": "DVE", "ACT": "Activation", "SP": "SP"}
    for inst in insts:
        si = inst.sync_info
        waits = [(w.ant_name, w.wait_value) for w in si.on_wait] if si else []
        if type(inst).__name__ == "InstDMACopy":
            # completion updates belong to the DMA lane proc
            for u in si.on_update:
                add_to_stream(inst, proc_of_sem(u.ant_name), waits, u.update_value)
        else:
            en = str(inst.engine).split(".")[-1]
            pref = eng_sem.get(en)
            if pref is None:
                continue
            upd = 0
            if si:
                for u in si.on_update:
                    if proc_of_sem(u.ant_name) == pref:
                        upd += u.update_value
            add_to_stream(inst, pref, waits, upd)

    from functools import lru_cache

    @lru_cache(maxsize=None)
    def holds(proc, tick, sem_d, v_d, depth=4):
        """Once `proc`'s sem has reached `tick`, does sem_d >= v_d hold?

        Covered prefix: entries up to the last one whose own completion is
        certified (cumulative sem value <= tick) have issued, so their waits
        held at some past moment; sems are monotone, so they hold now.
        """
        if proc == proc_of_sem(sem_d):
            return tick >= v_d
        if depth == 0 or not proc.startswith(_INORDER):
            return False
        stream = streams.get(proc, [])
        last = -1
        prev = 0
        for i, (waits, cum) in enumerate(stream):
            if cum > tick:
                break
            if cum > prev:
                last = i  # completing instruction within budget
            prev = cum
        for waits, _cum in stream[: last + 1]:
            for (s, v) in waits:
                if s == sem_d and v >= v_d:
                    return True
                if holds(proc_of_sem(s), v, sem_d, v_d, depth - 1):
                    return True
        return False

    for inst in insts:
        tn = type(inst).__name__
        si = inst.sync_info
        if si is None or len(si.on_wait) <= 1:
            continue
        # Drop waits implied by the instruction's own position in its
        # in-order stream(s): at least `v` completions of that proc precede
        # it in program order.
        own = [
            (proc, prefix)
            for proc, prefix in positions.get(id(inst), [])
            if proc.startswith(_INORDER)
        ]
        kept_sw = []
        for w in si.on_wait:
            wp = proc_of_sem(w.ant_name)
            if any(proc == wp and prefix >= w.wait_value for proc, prefix in own):
                continue
            kept_sw.append(w)
        if len(kept_sw) <= 1:
            si.on_wait = kept_sw
            continue
        waits = [(w.ant_name, w.wait_value) for w in kept_sw]
        chosen = None
        for k, (sem_k, v_k) in enumerate(waits):
            if not proc_of_sem(sem_k).startswith(_INORDER):
                continue
            if all(
                holds(proc_of_sem(sem_k), v_k, sem_d, v_d)
                for d, (sem_d, v_d) in enumerate(waits)
                if d != k
            ):
                chosen = k
                break
        assert chosen is not None, (
            f"{inst.name} ({tn}): cannot reduce waits to 1: {waits}"
        )
        si.on_wait = [kept_sw[chosen]]


def _host_prep(x, classifier_w, sel):
    """Compute perm / per-block weight-mask and the per-core bf16 shards."""
    x = np.asarray(x)
    w = np.asarray(classifier_w).astype(np.float32)
    sel = np.asarray(sel).astype(np.int64)

    w_abs = np.abs(w)
    idx = np.argsort(-w_abs, axis=1, kind="stable")  # matches jnp.argsort (stable)
    sig = (1.0 / (1.0 + np.exp(-w_abs.astype(np.float64)))).astype(np.float32)

    idx_sel = idx[sel]               # [C, CH]
    ch_ids = idx_sel[:, :G].T        # [G, C]
    perm = ch_ids.reshape(G * C)     # output channel g*C+c <- input channel
    wgh = sig[sel[None, :], ch_ids].reshape(G * C).astype(np.float64)

    # Per-channel-block weight/mask tile, with all scalar factors folded in:
    # wm[cb, i, j] = wgh_i * wgh_j * [same 16-group, j > i] / ((HW-1)*NUM_OFF*B)
    wm = np.zeros((N_CB, 128, 128), dtype=np.float64)
    scale = 1.0 / ((HW - 1) * NUM_OFF * B)
    ii, jj = np.meshgrid(np.arange(128), np.arange(128), indexing="ij")
    blockmask = ((ii // C) == (jj // C)) & (jj > ii)
    for cb in range(N_CB):
        wloc = wgh[cb * 128 : (cb + 1) * 128]
        wm[cb] = np.outer(wloc, wloc) * blockmask * scale
    wm = wm.astype(np.float32)

    # Per-core shards: samples [2c, 2c+1] -> permuted channels, hw-major,
    # pre-tiled as [s, slab, partition, k, c] so each partition's slab row is
    # one contiguous 8 KiB DRAM run.
    xr = x.reshape(B, CH, HW)
    shards = []
    for c in range(N_CORES):
        xs = xr[c * SAMPLES_PER_CORE : (c + 1) * SAMPLES_PER_CORE][:, perm, :]
        np_dt = mybir.dt.np(getattr(mybir.dt, DATA_DT_NAME))
        xb = xs.transpose(0, 2, 1).astype(np_dt)  # [S, HW, CH]
        xt = np.ascontiguousarray(
            xb.reshape(SAMPLES_PER_CORE, N_SLABS, SLAB, 128, CH).transpose(
                0, 1, 3, 2, 4
            )
        )
        shards.append(xt)
    return shards, wm


def kernel(x, classifier_w, sel):
    global _PROGRAM, LAST_RESULTS
    assert x.shape == (B, CH, H, W), x.shape

    shards, wm = _host_prep(x, classifier_w, sel)

    if _PROGRAM is None:
        _PROGRAM = _build_program()

    in_maps = [{"xt": shards[c], "wm": wm} for c in range(N_CORES)]
    LAST_RESULTS = run_bass_kernel_spmd(_PROGRAM, in_maps, core_ids=list(range(N_CORES)))

    total = np.float64(0.0)
    for r in LAST_RESULTS.results:
        total += np.float64(r["out"].sum(dtype=np.float64))
    return np.array([total], dtype=np.float32)


# revision 20
# speedup vs baseline: 1.0358x; 1.0062x over previous
"""Trainium2 Bass kernel for nn_SAW_53395033424216 (grouped-covariance loss).

Math (see reference): for each sample b and channel-group g (16 channels),
  cov[b,g] = (Xg Xg^T)/(HW-1) with Xg rows scaled by wgh; loss is the
  mean-over-B sum-over-g of the masked (strict upper triangle) abs-sum of
  cov / num_off.

Strategy:
  * Host: compute perm/wgh from classifier_w (tiny), permute channels so each
    group is 16 consecutive channels, transpose each sample to [HW, 512] and
    cast to bf16 (the 61k-entry abs-sum averages away bf16 noise; measured
    rel-err ~2.4e-6 on the fixed seed inputs).
  * Device (8 cores, 2 samples each): stream [128hw x 512ch] bf16 tiles;
    for each 128-channel block (= 8 whole groups) accumulate the 128x128
    Gram matrix over all 16384 hw rows via PE matmuls (contraction on
    partitions). Weight-scaling is bilinear -> folded into a per-block
    [128,128] mask/weight tile applied once at the end (DVE), followed by
    an abs row-reduce. Output: [128,1] partial sums per core.
  * Host: sum partials -> loss (clamp is a no-op since summands are >= 0).
"""

import os

# Whole-tile dependency tracking only: with per-subtile releases the slab DMA
# accumulates more sync-waits than the DMA pseudo-instruction format allows
# ("Too many sync wait commands" in walrus codegen).
os.environ.setdefault("BY_DEFAULT_DISABLE_SUBTILE_DEPS", "1")

import numpy as np
import ml_dtypes

import concourse.bass as bass
import concourse.mybir as mybir
from concourse.tile import TileContext
from concourse.bass_utils import run_bass_kernel_spmd

# Problem constants (hardcoded per the harness contract)
B = 16          # batch
CH = 512        # channels
H = W = 128
HW = H * W      # 16384
C = 16          # selected classes = group width
G = CH // C     # 32 groups
N_CORES = 8
SAMPLES_PER_CORE = B // N_CORES  # 2
NUM_OFF = C * (C - 1) // 2       # 120

# Data dtype on the wire/PE: bfloat16 (rel err ~2.4e-6) or float8_e4m3
# (rel err ~8.1e-4, half the DMA traffic).
DATA_DT_NAME = "float8e4"
SLAB = 4        # hw-chunks per DMA; small slabs let the first matmuls start early
N_WARMUP_MM = 52  # dummy matmuls during the initial DMA wait to lift the PE HAM throttle
N_CHUNKS = HW // 128             # 128
N_SLABS = N_CHUNKS // SLAB       # 16
N_CB = CH // 128                 # 4 channel blocks

_PROGRAM = None
LAST_RESULTS = None  # BassKernelResults of the most recent run (for test.py)


def _ensure_ntff_hook():
    """Provide antenv.axon_hooks if the image lacks it, so BASS_TRACE=1
    profiling works under axon (drives NTFF capture via the axon PJRT .so)."""
    try:
        import antenv.axon_hooks  # noqa: F401

        return
    except ImportError:
        pass
    import contextlib
    import ctypes
    import sys
    import types

    try:
        import antenv
    except ImportError:
        return

    so_path = "/opt/axon/libaxon_pjrt.so"
    if not os.path.exists(so_path):
        return
    lib = ctypes.CDLL(so_path)
    if not hasattr(lib, "axon_start_nrt_profile"):
        hook = None
    else:
        lib.axon_start_nrt_profile.argtypes = [
            ctypes.POINTER(ctypes.c_int64),
            ctypes.c_size_t,
        ]
        lib.axon_start_nrt_profile.restype = ctypes.c_int64
        lib.axon_stop_nrt_profile.argtypes = [ctypes.c_char_p]
        lib.axon_stop_nrt_profile.restype = ctypes.c_int64

        @contextlib.contextmanager
        def hook(output_dir, device_ids):
            import jax

            jax.devices()  # ensure the PJRT client exists before start
            if device_ids:
                ids = (ctypes.c_int64 * len(device_ids))(*device_ids)
                rc = lib.axon_start_nrt_profile(ids, len(device_ids))
            else:
                rc = lib.axon_start_nrt_profile(None, 0)
            if rc != 0:
                raise RuntimeError(f"axon_start_nrt_profile rc={rc}")
            try:
                yield
            finally:
                n = lib.axon_stop_nrt_profile(str(output_dir).encode())
                if n < 0:
                    raise RuntimeError(f"axon_stop_nrt_profile rc={n}")

    state = {"hook": hook}
    mod = types.ModuleType("antenv.axon_hooks")
    mod.get_axon_ntff_profile_hook = lambda: state["hook"]
    mod.set_axon_ntff_profile_hook = lambda h: state.update(hook=h)
    sys.modules["antenv.axon_hooks"] = mod
    antenv.axon_hooks = mod


_ensure_ntff_hook()


def _build_program():
    nc = bass.Bass()
    f32 = mybir.dt.float32
    data_dt = getattr(mybir.dt, DATA_DT_NAME)

    # Host pre-tiled layout: [s, slab, partition, k, c] so each partition's
    # slab slice is one contiguous 8 KiB run in DRAM (max DMA efficiency).
    xt = nc.dram_tensor(
        "xt", [SAMPLES_PER_CORE, N_SLABS, 128, SLAB, CH], data_dt, kind="ExternalInput"
    )
    wm = nc.dram_tensor("wm", [N_CB, 128, 128], f32, kind="ExternalInput")
    out = nc.dram_tensor("out", [128, 1], f32, kind="ExternalOutput")

    with TileContext(nc) as tc:
        with (
            tc.tile_pool(name="wpool", bufs=1) as wpool,
            tc.tile_pool(name="data", bufs=16) as dpool,
            tc.tile_pool(name="scratch", bufs=2) as spool,
            tc.tile_pool(name="redp", bufs=1) as redp,
            tc.tile_pool(name="psum", bufs=2, space="PSUM") as psum_pool,
        ):
            wm_t = wpool.tile([128, N_CB, 128], f32)
            nc.sync.dma_start(out=wm_t, in_=wm.transpose([1, 0, 2]))

            red_all = redp.tile([128, SAMPLES_PER_CORE * N_CB], f32)

            # PE warm-up: ~3us of throwaway matmuls while the first data slab
            # is still in flight, so the HAM clock gate reaches 8/8 before the
            # real stream begins.  Shares the gram0 slot tag; real use of that
            # slot starts with start=True which clears it.
            warm_in = wpool.tile([128, 128], data_dt, name="warm_in")
            nc.vector.memset(warm_in, 1)
            warm_ps = psum_pool.tile([128, 128], f32, name="warm_ps", tag="gram0")
            for _ in range(N_WARMUP_MM):
                nc.tensor.matmul(
                    warm_ps[:, :], lhsT=warm_in, rhs=warm_in, start=True, stop=True
                )

            for s in range(SAMPLES_PER_CORE):
                grams = [
                    psum_pool.tile(
                        [128, 128], f32, name=f"gram{cb}", tag=f"gram{cb}"
                    )
                    for cb in range(N_CB)
                ]
                for sl in range(N_SLABS):
                    dt_t = dpool.tile([128, SLAB, CH], data_dt)
                    nc.sync.dma_start(out=dt_t, in_=xt[s, sl])
                    for k in range(SLAB):
                        h = sl * SLAB + k
                        for cb in range(N_CB):
                            t = dt_t[:, k, cb * 128 : (cb + 1) * 128]
                            nc.tensor.matmul(
                                grams[cb][:, :],
                                lhsT=t,
                                rhs=t,
                                start=(h == 0),
                                stop=(h == N_CHUNKS - 1),
                            )
                # post-process: red[:, s*N_CB+cb] = sum_j |gram_ij| * wm[cb,i,j]
                for cb in range(N_CB):
                    scr = spool.tile([128, 128], f32)
                    nc.vector.tensor_mul(scr, grams[cb][:, :], wm_t[:, cb, :])
                    idx = s * N_CB + cb
                    nc.vector.tensor_reduce(
                        out=red_all[:, idx : idx + 1],
                        in_=scr,
                        axis=mybir.AxisListType.X,
                        op=mybir.AluOpType.add,
                        apply_absolute_value=True,
                    )

            racc = spool.tile([128, 1], f32, tag="racc")
            nc.vector.tensor_reduce(
                out=racc,
                in_=red_all,
                axis=mybir.AxisListType.X,
                op=mybir.AluOpType.add,
            )
            nc.gpsimd.dma_start(out=out[:, :], in_=racc, single_packet=True)

    _reduce_sync_waits(nc)
    return nc


# Procs whose semaphores advance in instruction (program) order.  DMA lanes
# qualify: each lane's DMAs go through the same FIFO ring and complete (inc
# their lane sem) in issue order per SDMA engine.  GpSimd (Pool) does not
# (8 independent Q7 FIFOs) - we never emit Pool work.
_INORDER = ("PE", "DVE", "Activation", "SP", "DMAHW", "DMASW")


def _reduce_sync_waits(nc):
    """Walrus' per-instruction sync-wait capacity is 1 for DMA/compute
    pseudo-instructions (and small for Drain), but Tile's semaphore pass is
    not transitively minimal and can emit more. Reduce every wait list to
    its weakest sufficient single wait by proving the rest redundant:

    (a) waits on the instruction's own in-order proc sem are implied by
        stream position;
    (b) for each candidate kept wait (sem_k >= v_k): every other wait
        (sem_d >= v_d) must hold once sem_k reaches v_k.  That holds if an
        instruction at-or-before tick v_k in sem_k's stream carried
        (transitively) a wait implying it -- sems are monotone, so a wait
        that held once holds forever.
    """
    insts = [i for fn in nc.m.functions for blk in fn.blocks for i in blk.instructions]

    def proc_of_sem(name):
        return name.rsplit("_", 1)[0]  # e.g. "DMAHW3_44" -> "DMAHW3"

    # Per proc: ordered stream of (waits, cumulative-sem-value-after).
    streams = {}
    # Per instruction id: [(proc, sem-value-before-this-instruction)]
    positions = {}

    def add_to_stream(inst, proc, waits, upd):
        lst = streams.setdefault(proc, [])
        prev = lst[-1][1] if lst else 0
        positions.setdefault(id(inst), []).append((proc, prev))
        lst.append((waits, prev + upd))

    eng_sem = {"PE": "PE", "DVE": "DVE", "ACT": "Activation", "SP": "SP"}
    for inst in insts:
        si = inst.sync_info
        waits = [(w.ant_name, w.wait_value) for w in si.on_wait] if si else []
        if type(inst).__name__ == "InstDMACopy":
            # completion updates belong to the DMA lane proc
            for u in si.on_update:
                add_to_stream(inst, proc_of_sem(u.ant_name), waits, u.update_value)
        else:
            en = str(inst.engine).split(".")[-1]
            pref = eng_sem.get(en)
            if pref is None:
                continue
            upd = 0
            if si:
                for u in si.on_update:
                    if proc_of_sem(u.ant_name) == pref:
                        upd += u.update_value
            add_to_stream(inst, pref, waits, upd)

    from functools import lru_cache

    @lru_cache(maxsize=None)
    def holds(proc, tick, sem_d, v_d, depth=4):
        """Once `proc`'s sem has reached `tick`, does sem_d >= v_d hold?

        Covered prefix: entries up to the last one whose own completion is
        certified (cumulative sem value <= tick) have issued, so their waits
        held at some past moment; sems are monotone, so they hold now.
        """
        if proc == proc_of_sem(sem_d):
            return tick >= v_d
        if depth == 0 or not proc.startswith(_INORDER):
            return False
        stream = streams.get(proc, [])
        last = -1
        prev = 0
        for i, (waits, cum) in enumerate(stream):
            if cum > tick:
                break
            if cum > prev:
                last = i  # completing instruction within budget
            prev = cum
        for waits, _cum in stream[: last + 1]:
            for (s, v) in waits:
                if s == sem_d and v >= v_d:
                    return True
                if holds(proc_of_sem(s), v, sem_d, v_d, depth - 1):
                    return True
        return False

    for inst in insts:
        tn = type(inst).__name__
        si = inst.sync_info
        if si is None or len(si.on_wait) <= 1:
            continue
        # Drop waits implied by the instruction's own position in its
        # in-order stream(s): at least `v` completions of that proc precede
        # it in program order.
        own = [
            (proc, prefix)
            for proc, prefix in positions.get(id(inst), [])
            if proc.startswith(_INORDER)
        ]
        kept_sw = []
        for w in si.on_wait:
            wp = proc_of_sem(w.ant_name)
            if any(proc == wp and prefix >= w.wait_value for proc, prefix in own):
                continue
            kept_sw.append(w)
        if len(kept_sw) <= 1:
            si.on_wait = kept_sw
            continue
        waits = [(w.ant_name, w.wait_value) for w in kept_sw]
        chosen = None
        for k, (sem_k, v_k) in enumerate(waits):
            if not proc_of_sem(sem_k).startswith(_INORDER):
                continue
            if all(
                holds(proc_of_sem(sem_k), v_k, sem_d, v_d)
                for d, (sem_d, v_d) in enumerate(waits)
                if d != k
            ):
                chosen = k
                break
        assert chosen is not None, (
            f"{inst.name} ({tn}): cannot reduce waits to 1: {waits}"
        )
        si.on_wait = [kept_sw[chosen]]


def _host_prep(x, classifier_w, sel):
    """Compute perm / per-block weight-mask and the per-core bf16 shards."""
    x = np.asarray(x)
    w = np.asarray(classifier_w).astype(np.float32)
    sel = np.asarray(sel).astype(np.int64)

    w_abs = np.abs(w)
    idx = np.argsort(-w_abs, axis=1, kind="stable")  # matches jnp.argsort (stable)
    sig = (1.0 / (1.0 + np.exp(-w_abs.astype(np.float64)))).astype(np.float32)

    idx_sel = idx[sel]               # [C, CH]
    ch_ids = idx_sel[:, :G].T        # [G, C]
    perm = ch_ids.reshape(G * C)     # output channel g*C+c <- input channel
    wgh = sig[sel[None, :], ch_ids].reshape(G * C).astype(np.float64)

    # Per-channel-block weight/mask tile, with all scalar factors folded in:
    # wm[cb, i, j] = wgh_i * wgh_j * [same 16-group, j > i] / ((HW-1)*NUM_OFF*B)
    wm = np.zeros((N_CB, 128, 128), dtype=np.float64)
    scale = 1.0 / ((HW - 1) * NUM_OFF * B)
    ii, jj = np.meshgrid(np.arange(128), np.arange(128), indexing="ij")
    blockmask = ((ii // C) == (jj // C)) & (jj > ii)
    for cb in range(N_CB):
        wloc = wgh[cb * 128 : (cb + 1) * 128]
        wm[cb] = np.outer(wloc, wloc) * blockmask * scale
    wm = wm.astype(np.float32)

    # Per-core shards: samples [2c, 2c+1] -> permuted channels, hw-major,
    # pre-tiled as [s, slab, partition, k, c] so each partition's slab row is
    # one contiguous 8 KiB DRAM run.
    xr = x.reshape(B, CH, HW)
    shards = []
    for c in range(N_CORES):
        xs = xr[c * SAMPLES_PER_CORE : (c + 1) * SAMPLES_PER_CORE][:, perm, :]
        np_dt = mybir.dt.np(getattr(mybir.dt, DATA_DT_NAME))
        xb = xs.transpose(0, 2, 1).astype(np_dt)  # [S, HW, CH]
        xt = np.ascontiguousarray(
            xb.reshape(SAMPLES_PER_CORE, N_SLABS, SLAB, 128, CH).transpose(
                0, 1, 3, 2, 4
            )
        )
        shards.append(xt)
    return shards, wm


def kernel(x, classifier_w, sel):
    global _PROGRAM, LAST_RESULTS
    assert x.shape == (B, CH, H, W), x.shape

    shards, wm = _host_prep(x, classifier_w, sel)

    if _PROGRAM is None:
        _PROGRAM = _build_program()

    in_maps = [{"xt": shards[c], "wm": wm} for c in range(N_CORES)]
    LAST_RESULTS = run_bass_kernel_spmd(_PROGRAM, in_maps, core_ids=list(range(N_CORES)))

    total = np.float64(0.0)
    for r in LAST_RESULTS.results:
        total += np.float64(r["out"].sum(dtype=np.float64))
    return np.array([total], dtype=np.float32)
